# revision 12
# baseline (speedup 1.0000x reference)
"""Multi-head attention Trainium2 kernel (8 NeuronCores, SPMD).

Problem: B=2, S=2048, E=1024, H=16, D=64 causal MHA with fp32 reference.

Sharding: core c handles batch b = c // 4 and heads [4*(c%4), 4*(c%4)+4).
Each core computes its 4 heads' Q/K/V projections, causal attention, and a
partial output projection against its rows of Wp.  The host sums the four
partials per batch and adds the bias.

v3 design (software-pipelined):
  - Activations are DMA'd in column halves so the first projections start
    ~6us in; Q/K/V projections for q-tile p+1 and the output projection
    for q-tile p-1 are issued as background units interleaved between
    attention blocks of q-tile p.  Engine queues are in-order, so proj
    units for tile p MUST drain before attention p issues (bg_proj); the
    out-proj units carry no forward deps (bg_out).
  - Softmax: one ACT spanning both s PSUM banks per block (probe-verified)
    with column-restricted causal blocks; a single [128,128] staircase
    multiply per partial block (deduped strips).
  - Normalization: reciprocal_approx_fast over the offset-0 accumulator
    (partition-offset sources are broken on HW for custom-DVE ops), f32r
    rounding copy, ones-matmul broadcast, lane-aligned multiplies; the
    s=1 half is partition-shifted into OTg[64:128] by a SBUF->SBUF DMA.
  - Output projection contracts full 128-partition head pairs.
"""

import sys
from collections import deque

import numpy as np

sys.path.insert(0, "/opt/trn_rl_repo")

import ml_dtypes  # noqa: E402
import concourse.bass as bass  # noqa: E402,F401
import concourse.tile as tile  # noqa: E402
from concourse import bacc, mybir  # noqa: E402
from concourse.bass_utils import run_bass_kernel_spmd  # noqa: E402

F32 = mybir.dt.float32
F32R = mybir.dt.float32r
BF16 = mybir.dt.bfloat16
EXP = mybir.ActivationFunctionType.Exp
COPY = mybir.ActivationFunctionType.Copy
BF = ml_dtypes.bfloat16

B, S, E, H, D = 2, 2048, 1024, 16, 64
N_CORES = 8
HC = H // 4          # heads per core (4)
EC = HC * D          # head cols per core (256)
QT = 512             # query tile (free dim of score matmuls)
KT = 128             # key tile (partition dim of score tiles)


def build_program(S=S, E=E, schedule=None, n_strips=0):
    nq = S // QT
    nk = S // KT
    nkc = E // 128   # contraction tiles for projections
    nm = S // 128    # m-tiles for V / output
    ne = E // 512    # e-tiles for output projection

    if schedule is None:
        schedule = [[(kj, 0, []) for kj in range(nk)] for _ in range(nq)]

    nc = bacc.Bacc(None, target_bir_lowering=False, debug=False)

    xqT = nc.dram_tensor("xqT", [E, S], BF16, kind="ExternalInput")
    xkT = nc.dram_tensor("xkT", [E, S], BF16, kind="ExternalInput")
    xvT = nc.dram_tensor("xvT", [E, S], BF16, kind="ExternalInput")
    wq = nc.dram_tensor("wq", [E, EC], BF16, kind="ExternalInput")
    wk = nc.dram_tensor("wk", [E, EC], BF16, kind="ExternalInput")
    wv = nc.dram_tensor("wv", [E, EC], BF16, kind="ExternalInput")
    wp = nc.dram_tensor("wp", [EC, E], BF16, kind="ExternalInput")
    mtd = None
    if n_strips:
        mtd = nc.dram_tensor("mtd", [n_strips * KT, KT], BF16,
                             kind="ExternalInput")
    outp = nc.dram_tensor("outp", [S, E], F32, kind="ExternalOutput")

    with tile.TileContext(nc) as tc:
        with (
            tc.tile_pool(name="const", bufs=1) as const,
            tc.tile_pool(name="big", bufs=1) as big,
            tc.tile_pool(name="pt", bufs=4) as ptp,
            tc.tile_pool(name="rd", bufs=2) as rdp,
            tc.tile_pool(name="bc", bufs=2) as bcp,
            tc.tile_pool(name="ot1", bufs=2) as ot1p,
            tc.tile_pool(name="osb", bufs=4) as osbp,
            tc.tile_pool(name="ps", bufs=1, space="PSUM") as psp,
        ):
            # ---- constants ----
            wq_sb = const.tile([128, nkc, EC], BF16, tag="wq")
            wk_sb = const.tile([128, nkc, EC], BF16, tag="wk")
            wv_sb = const.tile([128, nkc, EC], BF16, tag="wv")
            for w_sb, w in ((wq_sb, wq), (wk_sb, wk), (wv_sb, wv)):
                nc.sync.dma_start(
                    out=w_sb, in_=w.rearrange("(kc p) n -> p kc n", p=128))
            wpg_sb = []
            for g in range(2):
                t = const.tile([128, E], BF16, tag=f"wpg{g}",
                               name=f"wpg_sb{g}")
                nc.sync.dma_start(out=t, in_=wp[g * 128:(g + 1) * 128, :])
                wpg_sb.append(t)
            mt_sb = None
            if n_strips:
                mt_sb = const.tile([128, n_strips, KT], BF16, tag="mt")
                nc.sync.dma_start(
                    out=mt_sb,
                    in_=mtd.rearrange("(t p) q -> p t q", p=KT))
            ones_f = const.tile([128, 64], F32, tag="onesf")
            nc.vector.memset(ones_f, 1.0)
            ones_r = const.tile([128, 64], F32R, tag="onesr")
            nc.vector.tensor_copy(ones_r, ones_f)

            # ---- persistent intermediates ----
            QTg = [big.tile([128, S], BF16, tag=f"qt{g}", name=f"QTg{g}")
                   for g in range(2)]
            KTg = [big.tile([128, S], BF16, tag=f"kt{g}", name=f"KTg{g}")
                   for g in range(2)]
            vaug = big.tile([128, nm, HC, 65], BF16, tag="vaug")
            nc.vector.memset(vaug[:, :, :, 64:65], 1.0)
            OTg = [big.tile([128, S], BF16, tag=f"ot{g}", name=f"OTg{g}")
                   for g in range(2)]
            xfk = big.tile([128, nkc, S], BF16, tag="xfk")
            xfv = big.tile([128, nkc, S], BF16, tag="xfv")
            xfq = big.tile([128, nkc, S], BF16, tag="xfq")

            # ---- input DMAs: column halves, K then V then Q ----
            for half in range(2):
                cs = slice(half * (S // 2), (half + 1) * (S // 2))
                for xf, xT in ((xfk, xkT), (xfv, xvT), (xfq, xqT)):
                    for kc in range(nkc):
                        nc.sync.dma_start(
                            out=xf[:, kc, cs],
                            in_=xT[kc * 128:(kc + 1) * 128, cs])

            # ---- background unit definitions ----
            def kq_unit(w_sb, xf, dstg, mt, g):
                def run():
                    pss = psp.tile([128, 512], F32, tag="misc", bufs=2,
                                   name="pjps")
                    for kc in range(nkc):
                        nc.tensor.matmul(
                            pss, w_sb[:, kc, 128 * g:128 * (g + 1)],
                            xf[:, kc, mt * QT:(mt + 1) * QT],
                            start=(kc == 0), stop=(kc == nkc - 1))
                    nc.scalar.activation(
                        dstg[g][:, mt * QT:(mt + 1) * QT], pss, COPY)
                return run

            def v_unit(mt):
                def run():
                    psv = psp.tile([128, EC], F32, tag="misc", bufs=2,
                                   name="psv")
                    for kc in range(nkc):
                        nc.tensor.matmul(
                            psv, xfv[:, kc, mt * 128:(mt + 1) * 128],
                            wv_sb[:, kc, :],
                            start=(kc == 0), stop=(kc == nkc - 1))
                    nc.scalar.activation(
                        vaug[:, mt, :, 0:64],
                        psv.rearrange("p (h d) -> p h d", h=HC), COPY)
                return run

            def out_unit(mt, et):
                def run():
                    ps = psp.tile([128, 512], F32, tag="misc", bufs=2,
                                  name="ops")
                    for g in range(2):
                        nc.tensor.matmul(
                            ps, OTg[g][:, mt * 128:(mt + 1) * 128],
                            wpg_sb[g][:, et * 512:(et + 1) * 512],
                            start=(g == 0), stop=(g == 1))
                    osb = osbp.tile([128, 512], F32, tag="osb")
                    nc.vector.tensor_copy(osb, ps)
                    nc.sync.dma_start(
                        out=outp[mt * 128:(mt + 1) * 128,
                                 et * 512:(et + 1) * 512],
                        in_=osb)
                return run

            def proj_units(qi):
                us = []
                for g in range(2):
                    us.append(kq_unit(wk_sb, xfk, KTg, qi, g))
                for mt in range(4 * qi, 4 * qi + 4):
                    us.append(v_unit(mt))
                for g in range(2):
                    us.append(kq_unit(wq_sb, xfq, QTg, qi, g))
                return us

            bg_proj = deque()   # must drain before next attention tile
            bg_out = deque()    # no forward deps; drain anytime

            def pump():
                if bg_proj:
                    bg_proj.popleft()()
                elif bg_out:
                    bg_out.popleft()()

            # ---- prologue: q-tile 0 projections ----
            for u in proj_units(0):
                u()

            # ---- attention + interleaved background work ----
            for qi in range(nq):
                if qi + 1 < nq:
                    bg_proj.extend(proj_units(qi + 1))
                ks = schedule[qi]
                for g in range(2):
                    acc = [psp.tile([128, 512], F32, tag="ot", bufs=2,
                                    name=f"otps{s}")
                           for s in range(2)]
                    for idx, (kj, c0, strips) in enumerate(ks):
                        stp = psp.tile([128, 2, 512], F32, tag="stp",
                                       bufs=2, name="stp")
                        for s in range(2):
                            base = 64 * s
                            nc.tensor.matmul(
                                stp[:, s, c0:512],
                                KTg[g][base:base + 64,
                                       kj * KT:(kj + 1) * KT],
                                QTg[g][base:base + 64,
                                       qi * QT + c0:(qi + 1) * QT],
                                start=True, stop=True)
                        ptw = ptp.tile([128, 2, 512], BF16, tag="pt",
                                       name="ptw")
                        nc.scalar.activation(ptw[:, :, c0:512],
                                             stp[:, :, c0:512],
                                             EXP, scale=0.125)
                        for s in range(2):
                            for (coff, sidx) in strips:
                                nc.vector.tensor_mul(
                                    ptw[:, s, coff:coff + KT],
                                    ptw[:, s, coff:coff + KT],
                                    mt_sb[:, sidx, :])
                        for s in range(2):
                            h = 2 * g + s
                            nc.tensor.matmul(
                                acc[s][0:65, c0:512], vaug[:, kj, h, :],
                                ptw[:, s, c0:512],
                                start=(idx == 0), stop=(idx == len(ks) - 1))
                        pump()
                    # normalize this head-pair wave into OTg
                    rd = rdp.tile([65, 2, 512], F32, tag="rd")
                    rdr = rdp.tile([65, 2, 512], F32R, tag="rdr")
                    bc = bcp.tile([64, 2, 512], F32, tag="bc")
                    ot1 = ot1p.tile([64, 512], BF16, tag="ot1")
                    qs = slice(qi * QT, (qi + 1) * QT)
                    with nc.allow_low_precision(reason="softmax recip"):
                        for s in range(2):
                            nc.vector.reciprocal_approx_fast(
                                out=rd[0:65, s, :], in_=acc[s][0:65, :])
                        nc.vector.tensor_copy(rdr[64:65, :, :],
                                              rd[64:65, :, :])
                    for s in range(2):
                        bc_ps = psp.tile([64, 512], F32, tag="misc",
                                         bufs=2, name="bcps")
                        nc.tensor.matmul(
                            bc_ps, ones_r[64:65, :], rdr[64:65, s, :],
                            start=True, stop=True)
                        nc.vector.tensor_copy(bc[0:64, s, :], bc_ps)
                    nc.vector.tensor_mul(
                        OTg[g][0:64, qs], acc[0][0:64, :], bc[0:64, 0, :])
                    nc.vector.tensor_mul(
                        ot1, acc[1][0:64, :], bc[0:64, 1, :])
                    nc.sync.dma_start(out=OTg[g][64:128, qs], in_=ot1)
                for mt in range(4 * qi, 4 * (qi + 1)):
                    for et in range(ne):
                        bg_out.append(out_unit(mt, et))
                # leftover projections for qi+1 must issue before its
                # attention blocks hit the engine queues
                while bg_proj:
                    bg_proj.popleft()()
            while bg_out:
                bg_out.popleft()()

    nc.compile()
    return nc


def build_schedule(mask, S=S):
    """Classify (q-tile, k-tile) blocks from the actual mask content."""
    nq, nk = S // QT, S // KT
    schedule = []
    strips = []
    strip_key = {}
    for qi in range(nq):
        row = []
        for kj in range(nk):
            sub = mask[qi * QT:(qi + 1) * QT, kj * KT:(kj + 1) * KT]
            if not sub.any():
                continue
            vis = sub.any(axis=1)           # per-q-row visibility
            c0 = (int(np.argmax(vis)) // KT) * KT
            chunks = []
            for coff in range(c0, QT, KT):
                csub = sub[coff:coff + KT, :]
                if csub.all():
                    continue
                key = csub.tobytes()
                if key not in strip_key:
                    strip_key[key] = len(strips)
                    strips.append(
                        np.ascontiguousarray(csub.T).astype(BF))
                chunks.append((coff, strip_key[key]))
            row.append((kj, c0, tuple(chunks)))
        # blocks covering col 0 first so PSUM start=True zeroes the range
        row.sort(key=lambda t: t[1])
        assert not row or row[0][1] == 0, "first block must cover col 0"
        schedule.append(row)
    strip_blocks = (np.concatenate(strips, axis=0) if strips
                    else np.zeros((0, KT), BF))
    return schedule, strip_blocks


_CACHE = {}


def _get_program(sched_key, n_strips):
    if sched_key not in _CACHE:
        sched = [list(row) for row in sched_key]
        _CACHE[sched_key] = build_program(schedule=sched,
                                          n_strips=n_strips)
    return _CACHE[sched_key]


def kernel(xq, xk, xv, Wq, Wk, Wv, Wp, bp, mask, _trace=False):
    xq = np.asarray(xq, np.float32)
    xk = np.asarray(xk, np.float32)
    xv = np.asarray(xv, np.float32)
    Wq = np.asarray(Wq, np.float32)
    Wk = np.asarray(Wk, np.float32)
    Wv = np.asarray(Wv, np.float32)
    Wp = np.asarray(Wp, np.float32)
    bp = np.asarray(bp, np.float32)
    mask = np.asarray(mask)

    schedule, strip_blocks = build_schedule(mask)
    n_strips = strip_blocks.shape[0] // KT
    sched_key = tuple(tuple(row) for row in schedule)
    nc = _get_program(sched_key, n_strips)

    xT = {}
    for b in range(B):
        xT[("q", b)] = np.ascontiguousarray(xq[b].T).astype(BF)
        xT[("k", b)] = np.ascontiguousarray(xk[b].T).astype(BF)
        xT[("v", b)] = np.ascontiguousarray(xv[b].T).astype(BF)

    in_maps = []
    for c in range(N_CORES):
        b, hg = c // 4, c % 4
        cols = slice(EC * hg, EC * (hg + 1))
        m = {
            "xqT": xT[("q", b)],
            "xkT": xT[("k", b)],
            "xvT": xT[("v", b)],
            "wq": np.ascontiguousarray(Wq[:, cols]).astype(BF),
            "wk": np.ascontiguousarray(Wk[:, cols]).astype(BF),
            "wv": np.ascontiguousarray(Wv[:, cols]).astype(BF),
            "wp": np.ascontiguousarray(Wp[cols, :]).astype(BF),
        }
        if n_strips:
            m["mtd"] = strip_blocks
        in_maps.append(m)

    res = run_bass_kernel_spmd(nc, in_maps, core_ids=list(range(N_CORES)),
                               trace=_trace)
    out = np.zeros((B, S, E), np.float32)
    for c in range(N_CORES):
        out[c // 4] += res.results[c]["outp"]
    out += bp
    if _trace:
        kernel._last_results = res
    return out
